# revision 10
# baseline (speedup 1.0000x reference)
"""Trainium2 Bass kernel for nn_GSNN_83330955477864 (gnn_message_passing).

Contract: kernel(**inputs) takes the FULL (unsharded) inputs and returns the
FULL [B, N] float32 output, running the compute on 8 NeuronCores via
run_bass_kernel_spmd (data-parallel over the batch axis).

How this kernel works
---------------------
The reference network's output layer reads xl only at edges whose dst is an
output node.  For any such edge e, the per-layer edge update is

    e_l[:, e] = (sum_c h_l[:, src[e], c] * W3v[e, c]) * fnm[src[e]] + b3[e]
    xl_l      = (1-a)*(e_l + x0) + a*xl_{l-1},     x0 = x[:, src],  a = sigmoid(alpha)

so whenever fnm[src[e]] == 0 the entire node pipeline (scatter-add, batchnorm,
block-diagonal lin2, gather) is multiplicatively masked out of that edge and
the recurrence collapses to an affine gate driven only by x[:, src[e]] and
b3[e]; it telescopes to  xl_L = x0 + (1 - a^L) * b3.  kernel() performs that
backward-slice analysis on the host at build time from the actual index/mask
tensors it was handed.  When every output-feeding edge is closed-form (true
for this problem's graph: output edges' sources are output nodes, never
function nodes), the device kernel only needs

    out = coef * b3 + x,     coef = 1 - a^LAYERS

on the [B, K=1000] slab, which each of the 8 cores executes on its 1/8 batch
shard.  If the analysis ever found a non-closed-form output edge (not the
case for this graph family's deterministic setup), kernel() falls back to a
full numpy re-implementation of the reference.

Device program (per core): the host packs x-slab and b3 into one [BL=32, 2K]
buffer (per row/quarter: 250 x-values then 250 b3-values, each DMA row 2000
contiguous bytes); ONE SP-ring DMA fills a [128, 500] SBUF tile — a single
input DMA avoids the measured serialization of concurrent descriptor
generation in the shared HWDGE RTL; one DVE scalar_tensor_tensor produces
coef*b3 + x; the ACT ring stores the result slab.  Only the 1000 live
columns ever cross PCIe/HBM — the [B, N] output is assembled around zeros
on the host.
"""

import numpy as np

import concourse.bass as bass
import concourse.mybir as mybir
from concourse.bass_utils import run_bass_kernel_spmd

_RUNNER_CACHE = {}


def _make_cached_runner(nc):
    """Build a reusable jitted executable for `nc` (same lowering as
    bass2jax.run_bass_via_pjrt, but the jit closure is cached so repeat
    kernel() calls skip the per-call retrace + Bass->BIR recompile)."""
    import jax
    from jax.sharding import Mesh, PartitionSpec
    from jax.experimental.shard_map import shard_map
    from concourse import bass2jax

    bass2jax.install_neuronx_cc_hook()
    partition_name = nc.partition_id_tensor.name if nc.partition_id_tensor else None
    in_names, out_names, out_avals = [], [], []
    for alloc in nc.m.functions[0].allocations:
        if not isinstance(alloc, mybir.MemoryLocationSet):
            continue
        name = alloc.memorylocations[0].name
        if alloc.kind == "ExternalInput":
            if name != partition_name:
                in_names.append(name)
        elif alloc.kind == "ExternalOutput":
            out_names.append(name)
            out_avals.append(
                jax.core.ShapedArray(tuple(alloc.tensor_shape), mybir.dt.np(alloc.dtype))
            )
    n_params = len(in_names)
    zero_shapes = [(tuple(a.shape), a.dtype) for a in out_avals]
    all_in = list(in_names) + list(out_names)
    if partition_name is not None:
        all_in.append(partition_name)

    def _body(*args):
        operands = list(args)
        if partition_name is not None:
            operands.append(bass2jax.partition_id_tensor())
        return tuple(
            bass2jax._bass_exec_p.bind(
                *operands,
                out_avals=tuple(out_avals),
                in_names=tuple(all_in),
                out_names=tuple(out_names),
                lowering_input_output_aliases=(),
                sim_require_finite=True,
                sim_require_nnan=True,
                nc=nc,
            )
        )

    donate = tuple(range(n_params, n_params + len(out_avals)))
    devices = jax.devices()[:NCORES]
    mesh = Mesh(np.asarray(devices), ("core",))
    specs = (PartitionSpec("core"),)
    sharded = jax.jit(
        shard_map(
            _body, mesh=mesh,
            in_specs=specs * (n_params + len(out_avals)),
            out_specs=specs * len(out_names),
            check_rep=False,
        ),
        donate_argnums=donate,
        keep_unused=True,
    )

    def run(in_maps):
        concat = [
            np.ascontiguousarray(
                np.concatenate([np.asarray(m[n]) for m in in_maps], axis=0)
            )
            for n in in_names
        ]
        zeros = [np.zeros((NCORES * s[0], *s[1:]), d) for s, d in zero_shapes]
        outs = sharded(*concat, *zeros)
        return [
            {
                n: np.asarray(outs[i]).reshape(NCORES, *zero_shapes[i][0])[c]
                for i, n in enumerate(out_names)
            }
            for c in range(NCORES)
        ]

    return run

B, N, E, C, LAYERS = 256, 10000, 40000, 8, 4
EPS = 1e-5
NCORES = 8
BL = B // NCORES  # batch rows per core

# test.py pokes PROFILE for tracing; harness path leaves it alone.
PROFILE = False
LAST_RESULT = {}

_BUILD_CACHE = {}


def _is_iota(v):
    """v == v[0] + arange(len(v)) — contiguous ascending run."""
    v = np.asarray(v)
    return v.size > 0 and bool(np.all(v == v[0] + np.arange(v.size, dtype=v.dtype)))


def _np_reference(inputs):
    """Full float32 numpy mirror of the reference network (fallback path)."""
    x = np.asarray(inputs["x"], np.float32)
    ei = np.asarray(inputs["edge_index"])
    src, dst = ei[0].astype(np.int64), ei[1].astype(np.int64)
    fnm = np.asarray(inputs["function_node_mask"]).astype(np.float32)
    onm = np.asarray(inputs["output_node_mask"]).astype(bool)
    a = np.float32(1.0 / (1.0 + np.exp(-np.float64(np.asarray(inputs["alpha"]).reshape(())))))
    W1v = np.asarray(inputs["W1v"], np.float32)
    b1 = np.asarray(inputs["b1"], np.float32)
    g1 = np.asarray(inputs["gamma1"], np.float32)
    be1 = np.asarray(inputs["beta1"], np.float32)
    W2 = np.asarray(inputs["W2"], np.float32)
    b2 = np.asarray(inputs["b2"], np.float32)
    g2 = np.asarray(inputs["gamma2"], np.float32)
    be2 = np.asarray(inputs["beta2"], np.float32)
    W3v = np.asarray(inputs["W3v"], np.float32)
    b3 = np.asarray(inputs["b3"], np.float32)

    def bn(h, g, b):
        mu = h.mean(0)
        var = np.square(h - mu).mean(0)
        return (h - mu) / np.sqrt(var + EPS) * g + b

    def elu(v):
        return np.where(v > 0, v, np.expm1(np.minimum(v, 0)))

    Bsz = x.shape[0]
    x0 = x[:, src]
    xl = x0
    x_last = x0
    for _ in range(LAYERS):
        h = np.zeros((Bsz, N, C), np.float32)
        np.add.at(h, (slice(None), dst), xl[:, :, None] * W1v[None])
        h += b1
        h = elu(bn(h, g1, be1))
        h = np.einsum("bnc,ncd->bnd", h, W2) * fnm[None, :, None] + b2
        h = elu(bn(h, g2, be2))
        e = np.einsum("bec,ec->be", h[:, src], W3v) * fnm[src][None, :] + b3
        xl = (1 - a) * (e + x0) + a * x_last
        x_last = xl
    dst_mod = np.where(onm[dst], dst, N)
    out = np.zeros((Bsz, N + 1), np.float32)
    out[:, dst_mod] = xl  # unique real slots in practice; np last-wins otherwise
    return np.ascontiguousarray(out[:, :N])


def build_program(K, coef, repeats=1):
    """SPMD program for one core: out[BL,K] = coef*b3 + x[BL,K] slab.

    The host packs x and b3 into ONE input `xp` [BL, 2K]: for each batch row
    b and quarter q (J = K//4 columns each), xp holds [x(b,q) | b3(q)] as
    2J contiguous elements.  A single SP-ring DMA then fills the [128, 2J]
    SBUF tile (partition p = b*4 + q: free [0:J) = x, [J:2J) = b3) — one DMA
    instead of two, which matters because the HWDGE descriptor-generation
    RTL is TPB-level shared silicon: concurrent DGEs on the SP and ACT
    rings serialize (measured +0.3-0.5us).  One DVE scalar_tensor_tensor
    computes coef*b3 + x (fp16 inputs, f32 output: halves the input DMA's
    descriptor payload to 1000 B — still >= the 512 B full-rate threshold —
    and costs ~2e-4 relative error against the 2e-2 gate); the ACT ring
    stores the result slab.

    `repeats` > 1 re-runs the chain serially (each iteration's input DMA
    gated on the previous output DMA's completion) — used only by test.py's
    wall-clock slope measurement of the per-chain device latency.
    """
    J = K // 4
    f32 = mybir.dt.float32
    f16 = mybir.dt.float16

    nc = bass.Bass("TRN2", target_bir_lowering=False, debug=False)
    xp = nc.dram_tensor("xp", [BL, 2 * K], f16, kind="ExternalInput")
    outd = nc.dram_tensor("out", [BL, K], f32, kind="ExternalOutput")

    xp_ap = bass.AP(xp, 0, [[2 * K, BL], [1, 2 * K]])
    out_ap = bass.AP(outd, 0, [[K, BL], [J, 4], [1, J]])

    with (
        nc.sbuf_tensor("xb", [128, 2 * J], f16) as xb,
        nc.sbuf_tensor("ot", [128, J], f32) as ot,
        nc.semaphore("in_sem") as in_sem,
        nc.semaphore("vec_sem") as vec_sem,
        nc.semaphore("out_sem") as out_sem,
        nc.Block(no_gpsimd_drain=True) as block,
    ):

        @block.sync
        def _(sync):
            for i in range(repeats):
                if i > 0:
                    sync.wait_ge(out_sem, 16 * i)
                sync.dma_start(xb[:], xp_ap).then_inc(in_sem, 16)
            sync.wait_ge(out_sem, 16 * repeats)

        @block.scalar
        def _(scalar):
            for i in range(repeats):
                scalar.wait_ge(vec_sem, i + 1)
                scalar.dma_start(out_ap, ot[:]).then_inc(out_sem, 16)

        @block.vector
        def _(vector):
            for i in range(repeats):
                vector.wait_ge(in_sem, 16 * (i + 1))
                vector.scalar_tensor_tensor(
                    ot[:], xb[:, J : 2 * J], coef, xb[:, 0:J],
                    mybir.AluOpType.mult, mybir.AluOpType.add,
                ).then_inc(vec_sem, 1)

    return nc


def analyze(inputs):
    """Host-side backward slice from the output scatter.  Returns the slab
    descriptor (e0, s0, d0, K, coef) when the closed form applies, else None.
    """
    ei = np.asarray(inputs["edge_index"])
    src, dst = ei[0].astype(np.int64), ei[1].astype(np.int64)
    fnm = np.asarray(inputs["function_node_mask"]).astype(bool)
    onm = np.asarray(inputs["output_node_mask"]).astype(bool)
    alpha64 = float(np.asarray(inputs["alpha"]).reshape(()))

    oe = np.flatnonzero(onm[dst])  # edges written to real output slots
    closed_form = (
        oe.size > 0
        and oe.size % 4 == 0
        and np.unique(dst[oe]).size == oe.size  # one edge per output node
        and not fnm[src[oe]].any()  # lin3 masked out for every output edge
        and _is_iota(oe)  # b3 slab is one contiguous run
        and _is_iota(src[oe])  # x slab is one contiguous run
        and _is_iota(dst[oe])  # out slab is one contiguous run
    )
    if not closed_form:
        return None
    a = np.float32(1.0 / (1.0 + np.exp(-np.float64(alpha64))))
    coef = float(np.float32(1.0) - np.float32(a) ** np.int32(LAYERS))
    return int(oe[0]), int(src[oe[0]]), int(dst[oe[0]]), int(oe.size), coef


def kernel(**inputs) -> np.ndarray:
    x = np.asarray(inputs["x"], np.float32)
    b3 = np.asarray(inputs["b3"], np.float32)
    assert x.shape == (B, N) and b3.shape == (E,)

    desc = analyze(inputs)
    if desc is None:
        return _np_reference(inputs)
    e0, s0, d0, K, coef = desc

    key = (K, coef)
    if key not in _BUILD_CACHE:
        _BUILD_CACHE[key] = build_program(K, coef)
    nc = _BUILD_CACHE[key]

    # per-core input: this core's batch shard of the packed (x || b3) slab
    J = K // 4
    xpack = np.empty((B, 4, 2, J), np.float16)
    xpack[:, :, 0, :] = x[:, s0 : s0 + K].reshape(B, 4, J)
    xpack[:, :, 1, :] = b3[e0 : e0 + K].reshape(1, 4, J)
    xpack = xpack.reshape(B, 2 * K)
    in_maps = [
        {"xp": np.ascontiguousarray(xpack[k * BL : (k + 1) * BL])}
        for k in range(NCORES)
    ]

    if key in _RUNNER_CACHE:
        # repeat call: reuse the cached jitted executable (same NEFF)
        results = _RUNNER_CACHE[key](in_maps)
    else:
        try:
            res = run_bass_kernel_spmd(
                nc, in_maps, list(range(NCORES)), trace=bool(PROFILE)
            )
        except ModuleNotFoundError:
            # axon client without the NTFF profile hook: retry untraced
            res = run_bass_kernel_spmd(nc, in_maps, list(range(NCORES)), trace=False)
        if PROFILE:
            LAST_RESULT["exec_time_ns"] = res.exec_time_ns
            LAST_RESULT["profile_json"] = res.profile_json
            LAST_RESULT["instructions_and_trace"] = res.instructions_and_trace
        results = res.results
        try:
            _RUNNER_CACHE[key] = _make_cached_runner(nc)
        except Exception:
            pass  # repeat calls fall back to run_bass_kernel_spmd

    out = np.zeros((B, N), np.float32)
    out[:, d0 : d0 + K] = np.concatenate(
        [results[k]["out"] for k in range(NCORES)], axis=0
    )
    return out


# revision 11
# speedup vs baseline: 1.0454x; 1.0454x over previous
"""Trainium2 Bass kernel for nn_GSNN_83330955477864 (gnn_message_passing).

Contract: kernel(**inputs) takes the FULL (unsharded) inputs and returns the
FULL [B, N] float32 output, running the compute on 8 NeuronCores via
run_bass_kernel_spmd (data-parallel over the batch axis).

How this kernel works
---------------------
The reference network's output layer reads xl only at edges whose dst is an
output node.  For any such edge e, the per-layer edge update is

    e_l[:, e] = (sum_c h_l[:, src[e], c] * W3v[e, c]) * fnm[src[e]] + b3[e]
    xl_l      = (1-a)*(e_l + x0) + a*xl_{l-1},     x0 = x[:, src],  a = sigmoid(alpha)

so whenever fnm[src[e]] == 0 the entire node pipeline (scatter-add, batchnorm,
block-diagonal lin2, gather) is multiplicatively masked out of that edge and
the recurrence collapses to an affine gate driven only by x[:, src[e]] and
b3[e]; it telescopes to  xl_L = x0 + (1 - a^L) * b3.  kernel() performs that
backward-slice analysis on the host at build time from the actual index/mask
tensors it was handed.  When every output-feeding edge is closed-form (true
for this problem's graph: output edges' sources are output nodes, never
function nodes), the device kernel only needs

    out = coef * b3 + x,     coef = 1 - a^LAYERS

on the [B, K=1000] slab, which each of the 8 cores executes on its 1/8 batch
shard.  If the analysis ever found a non-closed-form output edge (not the
case for this graph family's deterministic setup), kernel() falls back to a
full numpy re-implementation of the reference.

Device program (per core): the host packs x-slab and b3 into one [BL=32, 2K]
buffer (per row/quarter: 250 x-values then 250 b3-values, each DMA row 2000
contiguous bytes); ONE SP-ring DMA fills a [128, 500] SBUF tile — a single
input DMA avoids the measured serialization of concurrent descriptor
generation in the shared HWDGE RTL; one DVE scalar_tensor_tensor produces
coef*b3 + x; the ACT ring stores the result slab.  Only the 1000 live
columns ever cross PCIe/HBM — the [B, N] output is assembled around zeros
on the host.  The Block skips the GpSimd DMA-queue drain at teardown
(no_gpsimd_drain=True — this kernel never touches the Q7), trimming the
NEFF's exit barrier; the explicit out_sem wait already fences the store.
"""

import numpy as np

import concourse.bass as bass
import concourse.mybir as mybir
from concourse.bass_utils import run_bass_kernel_spmd

_RUNNER_CACHE = {}


def _make_cached_runner(nc):
    """Build a reusable jitted executable for `nc` (same lowering as
    bass2jax.run_bass_via_pjrt, but the jit closure is cached so repeat
    kernel() calls skip the per-call retrace + Bass->BIR recompile)."""
    import jax
    from jax.sharding import Mesh, PartitionSpec
    from jax.experimental.shard_map import shard_map
    from concourse import bass2jax

    bass2jax.install_neuronx_cc_hook()
    partition_name = nc.partition_id_tensor.name if nc.partition_id_tensor else None
    in_names, out_names, out_avals = [], [], []
    for alloc in nc.m.functions[0].allocations:
        if not isinstance(alloc, mybir.MemoryLocationSet):
            continue
        name = alloc.memorylocations[0].name
        if alloc.kind == "ExternalInput":
            if name != partition_name:
                in_names.append(name)
        elif alloc.kind == "ExternalOutput":
            out_names.append(name)
            out_avals.append(
                jax.core.ShapedArray(tuple(alloc.tensor_shape), mybir.dt.np(alloc.dtype))
            )
    n_params = len(in_names)
    zero_shapes = [(tuple(a.shape), a.dtype) for a in out_avals]
    all_in = list(in_names) + list(out_names)
    if partition_name is not None:
        all_in.append(partition_name)

    def _body(*args):
        operands = list(args)
        if partition_name is not None:
            operands.append(bass2jax.partition_id_tensor())
        return tuple(
            bass2jax._bass_exec_p.bind(
                *operands,
                out_avals=tuple(out_avals),
                in_names=tuple(all_in),
                out_names=tuple(out_names),
                lowering_input_output_aliases=(),
                sim_require_finite=True,
                sim_require_nnan=True,
                nc=nc,
            )
        )

    donate = tuple(range(n_params, n_params + len(out_avals)))
    devices = jax.devices()[:NCORES]
    mesh = Mesh(np.asarray(devices), ("core",))
    specs = (PartitionSpec("core"),)
    sharded = jax.jit(
        shard_map(
            _body, mesh=mesh,
            in_specs=specs * (n_params + len(out_avals)),
            out_specs=specs * len(out_names),
            check_rep=False,
        ),
        donate_argnums=donate,
        keep_unused=True,
    )

    def run(in_maps):
        concat = [
            np.ascontiguousarray(
                np.concatenate([np.asarray(m[n]) for m in in_maps], axis=0)
            )
            for n in in_names
        ]
        zeros = [np.zeros((NCORES * s[0], *s[1:]), d) for s, d in zero_shapes]
        outs = sharded(*concat, *zeros)
        return [
            {
                n: np.asarray(outs[i]).reshape(NCORES, *zero_shapes[i][0])[c]
                for i, n in enumerate(out_names)
            }
            for c in range(NCORES)
        ]

    return run

B, N, E, C, LAYERS = 256, 10000, 40000, 8, 4
EPS = 1e-5
NCORES = 8
BL = B // NCORES  # batch rows per core

# test.py pokes PROFILE for tracing; harness path leaves it alone.
PROFILE = False
LAST_RESULT = {}

_BUILD_CACHE = {}


def _is_iota(v):
    """v == v[0] + arange(len(v)) — contiguous ascending run."""
    v = np.asarray(v)
    return v.size > 0 and bool(np.all(v == v[0] + np.arange(v.size, dtype=v.dtype)))


def _np_reference(inputs):
    """Full float32 numpy mirror of the reference network (fallback path)."""
    x = np.asarray(inputs["x"], np.float32)
    ei = np.asarray(inputs["edge_index"])
    src, dst = ei[0].astype(np.int64), ei[1].astype(np.int64)
    fnm = np.asarray(inputs["function_node_mask"]).astype(np.float32)
    onm = np.asarray(inputs["output_node_mask"]).astype(bool)
    a = np.float32(1.0 / (1.0 + np.exp(-np.float64(np.asarray(inputs["alpha"]).reshape(())))))
    W1v = np.asarray(inputs["W1v"], np.float32)
    b1 = np.asarray(inputs["b1"], np.float32)
    g1 = np.asarray(inputs["gamma1"], np.float32)
    be1 = np.asarray(inputs["beta1"], np.float32)
    W2 = np.asarray(inputs["W2"], np.float32)
    b2 = np.asarray(inputs["b2"], np.float32)
    g2 = np.asarray(inputs["gamma2"], np.float32)
    be2 = np.asarray(inputs["beta2"], np.float32)
    W3v = np.asarray(inputs["W3v"], np.float32)
    b3 = np.asarray(inputs["b3"], np.float32)

    def bn(h, g, b):
        mu = h.mean(0)
        var = np.square(h - mu).mean(0)
        return (h - mu) / np.sqrt(var + EPS) * g + b

    def elu(v):
        return np.where(v > 0, v, np.expm1(np.minimum(v, 0)))

    Bsz = x.shape[0]
    x0 = x[:, src]
    xl = x0
    x_last = x0
    for _ in range(LAYERS):
        h = np.zeros((Bsz, N, C), np.float32)
        np.add.at(h, (slice(None), dst), xl[:, :, None] * W1v[None])
        h += b1
        h = elu(bn(h, g1, be1))
        h = np.einsum("bnc,ncd->bnd", h, W2) * fnm[None, :, None] + b2
        h = elu(bn(h, g2, be2))
        e = np.einsum("bec,ec->be", h[:, src], W3v) * fnm[src][None, :] + b3
        xl = (1 - a) * (e + x0) + a * x_last
        x_last = xl
    dst_mod = np.where(onm[dst], dst, N)
    out = np.zeros((Bsz, N + 1), np.float32)
    out[:, dst_mod] = xl  # unique real slots in practice; np last-wins otherwise
    return np.ascontiguousarray(out[:, :N])


def build_program(K, coef, repeats=1):
    """SPMD program for one core: out[BL,K] = coef*b3 + x[BL,K] slab.

    The host packs x and b3 into ONE input `xp` [BL, 2K]: for each batch row
    b and quarter q (J = K//4 columns each), xp holds [x(b,q) | b3(q)] as
    2J contiguous elements.  A single SP-ring DMA then fills the [128, 2J]
    SBUF tile (partition p = b*4 + q: free [0:J) = x, [J:2J) = b3) — one DMA
    instead of two, which matters because the HWDGE descriptor-generation
    RTL is TPB-level shared silicon: concurrent DGEs on the SP and ACT
    rings serialize (measured +0.3-0.5us).  One DVE scalar_tensor_tensor
    computes coef*b3 + x (fp16 inputs, f32 output: halves the input DMA's
    descriptor payload to 1000 B — still >= the 512 B full-rate threshold —
    and costs ~2e-4 relative error against the 2e-2 gate); the ACT ring
    stores the result slab.

    `repeats` > 1 re-runs the chain serially (each iteration's input DMA
    gated on the previous output DMA's completion) — used only by test.py's
    wall-clock slope measurement of the per-chain device latency.
    """
    J = K // 4
    f32 = mybir.dt.float32
    f16 = mybir.dt.float16

    nc = bass.Bass("TRN2", target_bir_lowering=False, debug=False)
    xp = nc.dram_tensor("xp", [BL, 2 * K], f16, kind="ExternalInput")
    outd = nc.dram_tensor("out", [BL, K], f32, kind="ExternalOutput")

    xp_ap = bass.AP(xp, 0, [[2 * K, BL], [1, 2 * K]])
    out_ap = bass.AP(outd, 0, [[K, BL], [J, 4], [1, J]])

    with (
        nc.sbuf_tensor("xb", [128, 2 * J], f16) as xb,
        nc.sbuf_tensor("ot", [128, J], f32) as ot,
        nc.semaphore("in_sem") as in_sem,
        nc.semaphore("vec_sem") as vec_sem,
        nc.semaphore("out_sem") as out_sem,
        nc.Block(no_gpsimd_drain=True) as block,
    ):

        @block.sync
        def _(sync):
            for i in range(repeats):
                if i > 0:
                    sync.wait_ge(out_sem, 16 * i)
                sync.dma_start(xb[:], xp_ap).then_inc(in_sem, 16)
            sync.wait_ge(out_sem, 16 * repeats)

        @block.scalar
        def _(scalar):
            for i in range(repeats):
                scalar.wait_ge(vec_sem, i + 1)
                scalar.dma_start(out_ap, ot[:]).then_inc(out_sem, 16)

        @block.vector
        def _(vector):
            for i in range(repeats):
                vector.wait_ge(in_sem, 16 * (i + 1))
                vector.scalar_tensor_tensor(
                    ot[:], xb[:, J : 2 * J], coef, xb[:, 0:J],
                    mybir.AluOpType.mult, mybir.AluOpType.add,
                ).then_inc(vec_sem, 1)

    return nc


def analyze(inputs):
    """Host-side backward slice from the output scatter.  Returns the slab
    descriptor (e0, s0, d0, K, coef) when the closed form applies, else None.
    """
    ei = np.asarray(inputs["edge_index"])
    src, dst = ei[0].astype(np.int64), ei[1].astype(np.int64)
    fnm = np.asarray(inputs["function_node_mask"]).astype(bool)
    onm = np.asarray(inputs["output_node_mask"]).astype(bool)
    alpha64 = float(np.asarray(inputs["alpha"]).reshape(()))

    oe = np.flatnonzero(onm[dst])  # edges written to real output slots
    closed_form = (
        oe.size > 0
        and oe.size % 4 == 0
        and np.unique(dst[oe]).size == oe.size  # one edge per output node
        and not fnm[src[oe]].any()  # lin3 masked out for every output edge
        and _is_iota(oe)  # b3 slab is one contiguous run
        and _is_iota(src[oe])  # x slab is one contiguous run
        and _is_iota(dst[oe])  # out slab is one contiguous run
    )
    if not closed_form:
        return None
    a = np.float32(1.0 / (1.0 + np.exp(-np.float64(alpha64))))
    coef = float(np.float32(1.0) - np.float32(a) ** np.int32(LAYERS))
    return int(oe[0]), int(src[oe[0]]), int(dst[oe[0]]), int(oe.size), coef


def kernel(**inputs) -> np.ndarray:
    x = np.asarray(inputs["x"], np.float32)
    b3 = np.asarray(inputs["b3"], np.float32)
    assert x.shape == (B, N) and b3.shape == (E,)

    desc = analyze(inputs)
    if desc is None:
        return _np_reference(inputs)
    e0, s0, d0, K, coef = desc

    key = (K, coef)
    if key not in _BUILD_CACHE:
        _BUILD_CACHE[key] = build_program(K, coef)
    nc = _BUILD_CACHE[key]

    # per-core input: this core's batch shard of the packed (x || b3) slab
    J = K // 4
    xpack = np.empty((B, 4, 2, J), np.float16)
    xpack[:, :, 0, :] = x[:, s0 : s0 + K].reshape(B, 4, J)
    xpack[:, :, 1, :] = b3[e0 : e0 + K].reshape(1, 4, J)
    xpack = xpack.reshape(B, 2 * K)
    in_maps = [
        {"xp": np.ascontiguousarray(xpack[k * BL : (k + 1) * BL])}
        for k in range(NCORES)
    ]

    if key in _RUNNER_CACHE:
        # repeat call: reuse the cached jitted executable (same NEFF)
        results = _RUNNER_CACHE[key](in_maps)
    else:
        try:
            res = run_bass_kernel_spmd(
                nc, in_maps, list(range(NCORES)), trace=bool(PROFILE)
            )
        except ModuleNotFoundError:
            # axon client without the NTFF profile hook: retry untraced
            res = run_bass_kernel_spmd(nc, in_maps, list(range(NCORES)), trace=False)
        if PROFILE:
            LAST_RESULT["exec_time_ns"] = res.exec_time_ns
            LAST_RESULT["profile_json"] = res.profile_json
            LAST_RESULT["instructions_and_trace"] = res.instructions_and_trace
        results = res.results
        try:
            _RUNNER_CACHE[key] = _make_cached_runner(nc)
        except Exception:
            pass  # repeat calls fall back to run_bass_kernel_spmd

    out = np.zeros((B, N), np.float32)
    out[:, d0 : d0 + K] = np.concatenate(
        [results[k]["out"] for k in range(NCORES)], axis=0
    )
    return out
